# revision 6
# baseline (speedup 1.0000x reference)
"""Trainium2 Bass kernel for nn_ContinuousEmbedding (masked matmul + bias).

Computes out = x @ (weights * mask) + bias, reshaped to [B, in_size, out_size],
where mask zeroes each input feature's own [out_size]-wide diagonal block.

Strategy: tensor-parallel across the 8 NeuronCores by splitting the
in_size*out_size (=16384) output columns into 8 shards of 2048 columns.
The rel-err budget (2e-2) is large, so all matmul I/O is bf16: inputs are
cast on the host, the PE runs bf16 at full rate, and the output shard is
stored to HBM as bf16 (halving the dominant store traffic) then upcast on
the host.

Compute orientation is TRANSPOSED vs the torch view: each core computes
out_t[col, batch], i.e. matmul with lhsT = W[k, col_block] (stationary)
and rhs = x^T[k, batch] (moving).  That puts the io-columns on PSUM
partitions, so the bias becomes a per-partition scalar — eviction is a
1-op fused add+cast via tensor_scalar (DVE) / activation-Identity (ACT),
alternating between the two engines so eviction keeps up with the PE.
The host transposes the gathered [2048, 4096] shards back to [B, io].

All inputs (bias, masked W shard, x^T) are packed on the host into ONE
[128, 12304] bf16 "blob" laid out in EXACT consumption order, loaded by
9 DMA chunks sized so the first matmul's dependencies (bias + W m0 k0 +
first 512 of x^T) are only 168 KB — the real stream starts ~2.3us
earlier than with a monolithic first chunk.  A short PE warm-up covers
the DMA pipe-down and keeps the HAM clock-gate ramping to 8/8.

Tail: the last two col-blocks evict each 512-wide PSUM half as soon as
its stop-matmul retires (DVE takes s0, ACT takes s1) and store
per-group (per-half for the final group) so the ring drains with the
stream instead of piling 4 MB of stores after the last matmul.

Mask is constant — folded into the weights on the host.
"""

import numpy as np

B = 4096
IN_SIZE = 256
OUT_SIZE = 64
IO = IN_SIZE * OUT_SIZE          # 16384
N_CORES = 8
N_SHARD = IO // N_CORES          # 2048 output columns per core
P = 128                          # SBUF/PSUM partitions
KO = IN_SIZE // P                # 2 contraction sub-tiles
M_BLOCKS = N_SHARD // P          # 16 col-blocks per core
N_TILE = 512                     # matmul moving free dim (fp32 PSUM bank)
G_TILE = 1024                    # eviction group width (2 PSUM banks)
G_PER_M = B // G_TILE            # 4 groups per col-block
PSUM_BUFS = 4                    # 4 x 2 banks = all 8 PSUM banks
INTER = 4                        # col-blocks processed group-major first
WARM_MM = 26                     # PE warm-up matmuls (HAM un-throttle)
WARM_MID = 6                     # clock-keeper matmuls after col-block 0
TAIL_MM = 0                      # post-stream dummy matmuls (clock hold)

# ---- blob column layout (bf16 elements, strict consumption order) ----
OFF_BIAS = 0                                   # 16: bias_sw[p, m]
OFF_WM0K0 = OFF_BIAS + M_BLOCKS                # 16: W k0 m0
OFF_XT_G0K0 = OFF_WM0K0 + P                    # 144: xt k0 g0
OFF_WM0K1 = OFF_XT_G0K0 + G_TILE               # 1168: W k1 m0
OFF_XT_G0K1 = OFF_WM0K1 + P                    # 1296: xt k1 g0
OFF_W123 = OFF_XT_G0K1 + G_TILE                # 2320: W m1..3 (k0,k1)
OFF_XT_G1 = OFF_W123 + 3 * KO * P              # 3088: xt g1 (k0,k1)
OFF_XT_G2 = OFF_XT_G1 + KO * G_TILE            # 5136: xt g2
OFF_XT_G3 = OFF_XT_G2 + KO * G_TILE            # 7184: xt g3
OFF_W4 = OFF_XT_G3 + KO * G_TILE               # 9232: W m4..15
TOTAL = OFF_W4 + (M_BLOCKS - 4) * KO * P       # 12304

# Load chunks in consumption order; chunk 0 is just the first matmul's
# stationary weights, later chunks stream in just ahead of use.  Chunks
# alternate between the two HWDGE rings (sync/scalar) so doorbells go
# out two per ~0.65us while per-engine FIFO order stays consumption
# order.
CHUNKS = [
    (0, OFF_XT_G0K0),                          # bias + W m0 k0
    (OFF_XT_G0K0, OFF_XT_G0K0 + N_TILE),       # xt k0 s0
    (OFF_XT_G0K0 + N_TILE, OFF_WM0K1),         # xt k0 s1
    (OFF_WM0K1, OFF_XT_G0K1 + N_TILE),         # W m0 k1 + xt k1 s0
    (OFF_XT_G0K1 + N_TILE, OFF_W123),          # xt k1 s1
    (OFF_W123, OFF_XT_G1),                     # W m1..3
    (OFF_XT_G1, OFF_XT_G2),                    # xt g1
    (OFF_XT_G2, OFF_XT_G3),                    # xt g2
    (OFF_XT_G3, OFF_W4),                       # xt g3
    (OFF_W4, TOTAL),                           # W m4..15
]


def _w_off(k, m):
    if m == 0:
        return OFF_WM0K0 if k == 0 else OFF_WM0K1
    if m < 4:
        return OFF_W123 + (m - 1) * KO * P + k * P
    return OFF_W4 + (m - 4) * KO * P + k * P


def _xt_off(k, n):
    g, r = divmod(n, G_TILE)
    if g == 0:
        return (OFF_XT_G0K0 if k == 0 else OFF_XT_G0K1) + r
    base = {1: OFF_XT_G1, 2: OFF_XT_G2, 3: OFF_XT_G3}[g]
    return base + k * G_TILE + r


_CACHE: dict = {}


def _build_program():
    import concourse.mybir as mybir
    import concourse.tile as tile
    from concourse import bacc

    nc = bacc.Bacc(
        "TRN2", target_bir_lowering=False, debug=False, num_devices=N_CORES
    )
    bf16 = mybir.dt.bfloat16
    f32 = mybir.dt.float32
    blob = nc.dram_tensor("blob", [P, TOTAL], bf16, kind="ExternalInput").ap()
    # transposed output shard: out_t[col, batch]
    out = nc.dram_tensor("out", [N_SHARD, B], bf16, kind="ExternalOutput").ap()

    with tile.TileContext(nc) as tc:
        with tc.tile_pool(name="const", bufs=1) as const, \
             tc.tile_pool(name="psum", bufs=PSUM_BUFS, space="PSUM") as psum_pool, \
             tc.tile_pool(name="outp", bufs=6) as outp:
            blob_sb = const.tile([P, TOTAL], bf16)

            # Loads in consumption order, alternating between the two
            # HWDGE rings so issue (doorbell) serialization halves.
            ld = nc.sync
            for ci, (lo, hi) in enumerate(CHUNKS):
                eng = nc.sync if ci % 2 == 0 else nc.scalar
                eng.dma_start(out=blob_sb[:, lo:hi], in_=blob[:, lo:hi])

            # Warm-up while inputs stream in: short dummy matmuls keep the
            # PE busy until the first chunk lands so the HAM clock-gate is
            # at 8/8 (full rate) for the whole real stream; a dummy
            # activation pulls the ACT function table in early.  memsets
            # run on the otherwise-idle GpSimd so the first warm-up
            # matmul issues as soon as the engine prologue ends.
            warm_w = const.tile([P, P], bf16)
            warmf = const.tile([1, 1], f32)
            nc.gpsimd.memset(warm_w, 0.0)
            nc.gpsimd.memset(warmf, 0.0)
            nc.scalar.add(warmf[:], warmf[:], warmf[0:1, 0:1])
            # Unpack the packed bf16 bias columns to f32 (DVE scalar
            # operands must be f32).
            bias_sb = const.tile([P, M_BLOCKS], f32)
            nc.vector.tensor_copy(bias_sb[:], blob_sb[:, 0:M_BLOCKS])
            warm_ps = psum_pool.tile([P, G_TILE], f32, name="warm_ps", tag="ps")
            for _ in range(WARM_MM):
                nc.tensor.matmul(
                    warm_ps[:, 0:P], lhsT=warm_w[:], rhs=warm_w[:],
                    start=True, stop=True,
                )

            # Execution order: group-major over the first INTER col-blocks
            # (so full x^T is only needed after ~16 groups), then
            # block-major for the rest.
            order = [(m, g) for g in range(G_PER_M) for m in range(INTER)]
            order += [(m, g) for m in range(INTER, M_BLOCKS)
                      for g in range(G_PER_M)]
            out_sbs = {}
            for pos, (m, g) in enumerate(order):
                ms = slice(m * P, (m + 1) * P)
                if m not in out_sbs:
                    out_sbs[m] = outp.tile([P, B], bf16, name=f"osb{m}",
                                           tag="osb")
                out_sb = out_sbs[m]
                ps = psum_pool.tile([P, G_TILE], f32, name=f"ps{m}_{g}",
                                    tag="ps")
                for k in range(KO):
                    wof = _w_off(k, m)
                    for s in range(G_TILE // N_TILE):
                        n0 = g * G_TILE + s * N_TILE
                        xof = _xt_off(k, n0)
                        nc.tensor.matmul(
                            ps[:, s * N_TILE:(s + 1) * N_TILE],
                            lhsT=blob_sb[:, wof:wof + P],
                            rhs=blob_sb[:, xof:xof + N_TILE],
                            start=(k == 0),
                            stop=(k == KO - 1),
                        )
                if pos == 0:
                    # A few clock-keeper matmuls between the first and
                    # second col-block: the load stream is still
                    # catching up here and a PE stall would reset the
                    # HAM ramp.
                    for _ in range(WARM_MID):
                        nc.tensor.matmul(
                            warm_ps[:, 0:P], lhsT=warm_w[:], rhs=warm_w[:],
                            start=True, stop=True,
                        )
                gs = slice(g * G_TILE, (g + 1) * G_TILE)
                if m == M_BLOCKS - 1 and g == G_PER_M - 1:
                    # Very last group: evict each 512-wide half as soon
                    # as its stop-matmul retires (s0 stops one matmul
                    # before s1), split across both engines, and store
                    # per half so the last bytes leave ASAP.
                    h = G_TILE // 2
                    nc.vector.tensor_scalar_add(
                        out_sb[:, gs.start:gs.start + h],
                        ps[:, 0:h], bias_sb[:, m:m + 1]
                    )
                    nc.scalar.add(
                        out_sb[:, gs.start + h:gs.stop],
                        ps[:, h:], bias_sb[:, m:m + 1]
                    )
                    ld.dma_start(out=out[ms, gs.start:gs.start + h],
                                 in_=out_sb[:, gs.start:gs.start + h])
                    ld.dma_start(out=out[ms, gs.start + h:gs.stop],
                                 in_=out_sb[:, gs.start + h:gs.stop])
                    continue
                if pos % 2 == 0:
                    nc.vector.tensor_scalar_add(
                        out_sb[:, gs], ps[:], bias_sb[:, m:m + 1]
                    )
                else:
                    nc.scalar.add(out_sb[:, gs], ps[:], bias_sb[:, m:m + 1])
                if m >= M_BLOCKS - 2:
                    # Last two col-blocks: store per group so the ring
                    # drains with the stream instead of piling 4 MB of
                    # stores after the last matmul.
                    ld.dma_start(out=out[ms, gs], in_=out_sb[:, gs])
                elif g % 2 == 1:
                    # Stores per batch-half (512 KB each, smooth stream).
                    hs = slice((g - 1) * G_TILE, (g + 1) * G_TILE)
                    ld.dma_start(out=out[ms, hs], in_=out_sb[:, hs])

            # Optional post-stream dummy matmuls: hold the HAM clock at
            # 8/8 while the teardown sweep runs.
            for _ in range(TAIL_MM):
                nc.tensor.matmul(
                    warm_ps[:, 0:P], lhsT=warm_w[:], rhs=warm_w[:],
                    start=True, stop=True,
                )

    nc.compile()
    return nc


def _get_program():
    if "prog" not in _CACHE:
        _CACHE["prog"] = _build_program()
    return _CACHE["prog"]


def _shard_inputs(x, weights, bias):
    import ml_dtypes

    bf16 = ml_dtypes.bfloat16
    # Fold the constant block-diagonal mask into the weights on the host.
    col_block = np.arange(IO, dtype=np.int64) // OUT_SIZE
    mask = (col_block[None, :] != np.arange(IN_SIZE)[:, None])
    wm = (weights * mask.astype(weights.dtype)).astype(bf16)
    xt = np.ascontiguousarray(x.T.astype(bf16))
    in_maps = []
    for c in range(N_CORES):
        sl = slice(c * N_SHARD, (c + 1) * N_SHARD)
        ws = wm[:, sl]
        blob = np.empty((P, TOTAL), dtype=bf16)
        blob[:, OFF_BIAS:OFF_BIAS + M_BLOCKS] = \
            bias[sl].reshape(M_BLOCKS, P).T.astype(bf16)
        for k in range(KO):
            kr = slice(k * P, (k + 1) * P)
            for m in range(M_BLOCKS):
                blob[:, _w_off(k, m):_w_off(k, m) + P] = \
                    ws[kr, m * P:(m + 1) * P]
            for g in range(G_PER_M):
                blob[:, _xt_off(k, g * G_TILE):_xt_off(k, g * G_TILE) +
                     G_TILE] = xt[kr, g * G_TILE:(g + 1) * G_TILE]
        in_maps.append({"blob": blob})
    return in_maps


def run_sharded(in_maps, **kwargs):
    """Run the SPMD program on cores 0-7. kwargs forwarded (e.g. trace)."""
    from concourse.bass_utils import run_bass_kernel_spmd

    nc = _get_program()
    return run_bass_kernel_spmd(
        nc, in_maps, core_ids=list(range(N_CORES)), **kwargs
    )


def kernel(x: np.ndarray, weights: np.ndarray, bias: np.ndarray) -> np.ndarray:
    x = np.asarray(x, dtype=np.float32)
    weights = np.asarray(weights, dtype=np.float32)
    bias = np.asarray(bias, dtype=np.float32)
    in_maps = _shard_inputs(x, weights, bias)
    res = run_sharded(in_maps)
    full = np.empty((B, IO), dtype=np.float32)
    for c in range(N_CORES):
        sl = slice(c * N_SHARD, (c + 1) * N_SHARD)
        full[:, sl] = np.asarray(res.results[c]["out"]).astype(np.float32).T
    return full.reshape(B, IN_SIZE, OUT_SIZE)


# revision 10
# speedup vs baseline: 1.0709x; 1.0709x over previous
"""Trainium2 Bass kernel for nn_ContinuousEmbedding (masked matmul + bias).

Computes out = x @ (weights * mask) + bias, reshaped to [B, in_size, out_size],
where mask zeroes each input feature's own [out_size]-wide diagonal block.

Strategy: tensor-parallel across the 8 NeuronCores by splitting the
in_size*out_size (=16384) output columns into 8 shards of 2048 columns.
The rel-err budget (2e-2) is large, so all matmul I/O is bf16: inputs are
cast on the host, the PE runs bf16 at full rate, and the output shard is
stored to HBM as bf16 (halving the dominant store traffic) then upcast on
the host.

Compute orientation is TRANSPOSED vs the torch view: each core computes
out_t[col, batch], i.e. matmul with lhsT = W[k, col_block] (stationary)
and rhs = x^T[k, batch] (moving).  That puts the io-columns on PSUM
partitions, so the bias becomes a per-partition scalar — eviction is a
1-op fused add+cast via tensor_scalar (DVE) / activation-Identity (ACT),
alternating between the two engines so eviction keeps up with the PE.
The host transposes the gathered [2048, 4096] shards back to [B, io].

All inputs (bias, masked W shard, x^T) are packed on the host into ONE
[128, 12304] bf16 "blob" laid out in EXACT consumption order, loaded by
9 DMA chunks sized so the first matmul's dependencies (bias + W m0 k0 +
first 512 of x^T) are only 168 KB — the real stream starts ~2.3us
earlier than with a monolithic first chunk.  A short PE warm-up covers
the DMA pipe-down and keeps the HAM clock-gate ramping to 8/8.

Tail: the last two col-blocks evict each 512-wide PSUM half as soon as
its stop-matmul retires (DVE takes s0, ACT takes s1) and store
per-group (per-half for the final group) so the ring drains with the
stream instead of piling 4 MB of stores after the last matmul.

Mask is constant — folded into the weights on the host.
"""

import numpy as np

B = 4096
IN_SIZE = 256
OUT_SIZE = 64
IO = IN_SIZE * OUT_SIZE          # 16384
N_CORES = 8
N_SHARD = IO // N_CORES          # 2048 output columns per core
P = 128                          # SBUF/PSUM partitions
KO = IN_SIZE // P                # 2 contraction sub-tiles
M_BLOCKS = N_SHARD // P          # 16 col-blocks per core
N_TILE = 512                     # matmul moving free dim (fp32 PSUM bank)
G_TILE = 1024                    # eviction group width (2 PSUM banks)
G_PER_M = B // G_TILE            # 4 groups per col-block
PSUM_BUFS = 4                    # 4 x 2 banks = all 8 PSUM banks
INTER = 4                        # col-blocks processed group-major first
WARM_MM = 26                     # PE warm-up matmuls (HAM un-throttle)
TAIL_MM = 0                      # post-stream dummy matmuls (clock hold)

# ---- blob column layout (bf16 elements, strict consumption order) ----
OFF_BIAS = 0                                   # 16: bias_sw[p, m]
OFF_WM0K0 = OFF_BIAS + M_BLOCKS                # 16: W k0 m0
OFF_XT_G0K0 = OFF_WM0K0 + P                    # 144: xt k0 g0
OFF_WM0K1 = OFF_XT_G0K0 + G_TILE               # 1168: W k1 m0
OFF_XT_G0K1 = OFF_WM0K1 + P                    # 1296: xt k1 g0
OFF_W123 = OFF_XT_G0K1 + G_TILE                # 2320: W m1..3 (k0,k1)
OFF_XT_G1 = OFF_W123 + 3 * KO * P              # 3088: xt g1 (k0,k1)
OFF_XT_G2 = OFF_XT_G1 + KO * G_TILE            # 5136: xt g2
OFF_XT_G3 = OFF_XT_G2 + KO * G_TILE            # 7184: xt g3
OFF_W4 = OFF_XT_G3 + KO * G_TILE               # 9232: W m4..15
TOTAL = OFF_W4 + (M_BLOCKS - 4) * KO * P       # 12304

# Load chunks in consumption order; chunk 0 is just the first matmul's
# stationary weights, later chunks stream in just ahead of use.  Chunks
# alternate between the two HWDGE rings (sync/scalar) so doorbells go
# out two per ~0.65us while per-engine FIFO order stays consumption
# order.
CHUNKS = [
    (0, OFF_XT_G0K0),                          # bias + W m0 k0
    (OFF_XT_G0K0, OFF_XT_G0K0 + N_TILE),       # xt k0 s0
    (OFF_XT_G0K0 + N_TILE, OFF_WM0K1),         # xt k0 s1
    (OFF_WM0K1, OFF_XT_G0K1 + N_TILE),         # W m0 k1 + xt k1 s0
    (OFF_XT_G0K1 + N_TILE, OFF_W123),          # xt k1 s1
    (OFF_W123, OFF_XT_G1),                     # W m1..3
    (OFF_XT_G1, OFF_XT_G2),                    # xt g1
    (OFF_XT_G2, OFF_XT_G3),                    # xt g2
    (OFF_XT_G3, OFF_W4),                       # xt g3
    (OFF_W4, TOTAL),                           # W m4..15
]

# Clock-keeper fills (PE dummy matmuls) emitted while the load stream
# is still catching up, keyed by (pos, k) emission point: a PE stall in
# this window resets the HAM clock ramp, so fill instead of stalling.
FILLS = {(0, 1): 4, (1, 0): 4, (2, 0): 2}


def _w_off(k, m):
    if m == 0:
        return OFF_WM0K0 if k == 0 else OFF_WM0K1
    if m < 4:
        return OFF_W123 + (m - 1) * KO * P + k * P
    return OFF_W4 + (m - 4) * KO * P + k * P


def _xt_off(k, n):
    g, r = divmod(n, G_TILE)
    if g == 0:
        return (OFF_XT_G0K0 if k == 0 else OFF_XT_G0K1) + r
    base = {1: OFF_XT_G1, 2: OFF_XT_G2, 3: OFF_XT_G3}[g]
    return base + k * G_TILE + r


_CACHE: dict = {}


def _build_program():
    import concourse.mybir as mybir
    import concourse.tile as tile
    from concourse import bacc

    nc = bacc.Bacc(
        "TRN2", target_bir_lowering=False, debug=False, num_devices=N_CORES
    )
    bf16 = mybir.dt.bfloat16
    f32 = mybir.dt.float32
    blob = nc.dram_tensor("blob", [P, TOTAL], bf16, kind="ExternalInput").ap()
    # transposed output shard: out_t[col, batch]
    out = nc.dram_tensor("out", [N_SHARD, B], bf16, kind="ExternalOutput").ap()

    with tile.TileContext(nc) as tc:
        with tc.tile_pool(name="const", bufs=1) as const, \
             tc.tile_pool(name="psum", bufs=PSUM_BUFS, space="PSUM") as psum_pool, \
             tc.tile_pool(name="outp", bufs=6) as outp:
            blob_sb = const.tile([P, TOTAL], bf16)

            # Loads in consumption order on a single HWDGE ring (the
            # second ring turned out to be heavily throttled).
            ld = nc.sync
            for lo, hi in CHUNKS:
                ld.dma_start(out=blob_sb[:, lo:hi], in_=blob[:, lo:hi])

            # Warm-up while inputs stream in: short dummy matmuls keep the
            # PE busy until the first chunk lands so the HAM clock-gate is
            # at 8/8 (full rate) for the whole real stream; a dummy
            # activation pulls the ACT function table in early.  memsets
            # run on the otherwise-idle GpSimd so the first warm-up
            # matmul issues as soon as the engine prologue ends.
            warm_w = const.tile([P, P], bf16)
            warmf = const.tile([1, 1], f32)
            nc.gpsimd.memset(warm_w, 0.0)
            nc.gpsimd.memset(warmf, 0.0)
            nc.scalar.add(warmf[:], warmf[:], warmf[0:1, 0:1])
            # Unpack the packed bf16 bias columns to f32 (DVE scalar
            # operands must be f32).
            bias_sb = const.tile([P, M_BLOCKS], f32)
            nc.vector.tensor_copy(bias_sb[:], blob_sb[:, 0:M_BLOCKS])
            warm_ps = psum_pool.tile([P, G_TILE], f32, name="warm_ps", tag="ps")
            for _ in range(WARM_MM):
                nc.tensor.matmul(
                    warm_ps[:, 0:P], lhsT=warm_w[:], rhs=warm_w[:],
                    start=True, stop=True,
                )

            # Execution order: group-major over the first INTER col-blocks
            # (so full x^T is only needed after ~16 groups), then
            # block-major for the rest.
            order = [(m, g) for g in range(G_PER_M) for m in range(INTER)]
            order += [(m, g) for m in range(INTER, M_BLOCKS)
                      for g in range(G_PER_M)]
            out_sbs = {}
            for pos, (m, g) in enumerate(order):
                ms = slice(m * P, (m + 1) * P)
                if m not in out_sbs:
                    out_sbs[m] = outp.tile([P, B], bf16, name=f"osb{m}",
                                           tag="osb")
                out_sb = out_sbs[m]
                ps = psum_pool.tile([P, G_TILE], f32, name=f"ps{m}_{g}",
                                    tag="ps")
                for k in range(KO):
                    for _ in range(FILLS.get((pos, k), 0)):
                        nc.tensor.matmul(
                            warm_ps[:, 0:P], lhsT=warm_w[:], rhs=warm_w[:],
                            start=True, stop=True,
                        )
                    wof = _w_off(k, m)
                    for s in range(G_TILE // N_TILE):
                        n0 = g * G_TILE + s * N_TILE
                        xof = _xt_off(k, n0)
                        nc.tensor.matmul(
                            ps[:, s * N_TILE:(s + 1) * N_TILE],
                            lhsT=blob_sb[:, wof:wof + P],
                            rhs=blob_sb[:, xof:xof + N_TILE],
                            start=(k == 0),
                            stop=(k == KO - 1),
                        )
                gs = slice(g * G_TILE, (g + 1) * G_TILE)
                if m == M_BLOCKS - 1 and g == G_PER_M - 1:
                    # Very last group: evict each 512-wide half as soon
                    # as its stop-matmul retires (s0 stops one matmul
                    # before s1), split across both engines, and store
                    # per half so the last bytes leave ASAP.
                    h = G_TILE // 2
                    nc.vector.tensor_scalar_add(
                        out_sb[:, gs.start:gs.start + h],
                        ps[:, 0:h], bias_sb[:, m:m + 1]
                    )
                    nc.scalar.add(
                        out_sb[:, gs.start + h:gs.stop],
                        ps[:, h:], bias_sb[:, m:m + 1]
                    )
                    ld.dma_start(out=out[ms, gs.start:gs.start + h],
                                 in_=out_sb[:, gs.start:gs.start + h])
                    ld.dma_start(out=out[ms, gs.start + h:gs.stop],
                                 in_=out_sb[:, gs.start + h:gs.stop])
                    continue
                if pos % 2 == 0:
                    nc.vector.tensor_scalar_add(
                        out_sb[:, gs], ps[:], bias_sb[:, m:m + 1]
                    )
                else:
                    nc.scalar.add(out_sb[:, gs], ps[:], bias_sb[:, m:m + 1])
                if m >= M_BLOCKS - 2:
                    # Last two col-blocks: store per group so the ring
                    # drains with the stream instead of piling 4 MB of
                    # stores after the last matmul.
                    ld.dma_start(out=out[ms, gs], in_=out_sb[:, gs])
                elif g % 2 == 1:
                    # Stores per batch-half (512 KB each, smooth stream).
                    hs = slice((g - 1) * G_TILE, (g + 1) * G_TILE)
                    ld.dma_start(out=out[ms, hs], in_=out_sb[:, hs])

            # Optional post-stream dummy matmuls: hold the HAM clock at
            # 8/8 while the teardown sweep runs.
            for _ in range(TAIL_MM):
                nc.tensor.matmul(
                    warm_ps[:, 0:P], lhsT=warm_w[:], rhs=warm_w[:],
                    start=True, stop=True,
                )

    nc.compile()
    return nc


def _get_program():
    if "prog" not in _CACHE:
        _CACHE["prog"] = _build_program()
    return _CACHE["prog"]


def _shard_inputs(x, weights, bias):
    import ml_dtypes

    bf16 = ml_dtypes.bfloat16
    # Fold the constant block-diagonal mask into the weights on the host.
    col_block = np.arange(IO, dtype=np.int64) // OUT_SIZE
    mask = (col_block[None, :] != np.arange(IN_SIZE)[:, None])
    wm = (weights * mask.astype(weights.dtype)).astype(bf16)
    xt = np.ascontiguousarray(x.T.astype(bf16))
    in_maps = []
    for c in range(N_CORES):
        sl = slice(c * N_SHARD, (c + 1) * N_SHARD)
        ws = wm[:, sl]
        blob = np.empty((P, TOTAL), dtype=bf16)
        blob[:, OFF_BIAS:OFF_BIAS + M_BLOCKS] = \
            bias[sl].reshape(M_BLOCKS, P).T.astype(bf16)
        for k in range(KO):
            kr = slice(k * P, (k + 1) * P)
            for m in range(M_BLOCKS):
                blob[:, _w_off(k, m):_w_off(k, m) + P] = \
                    ws[kr, m * P:(m + 1) * P]
            for g in range(G_PER_M):
                blob[:, _xt_off(k, g * G_TILE):_xt_off(k, g * G_TILE) +
                     G_TILE] = xt[kr, g * G_TILE:(g + 1) * G_TILE]
        in_maps.append({"blob": blob})
    return in_maps


def run_sharded(in_maps, **kwargs):
    """Run the SPMD program on cores 0-7. kwargs forwarded (e.g. trace)."""
    from concourse.bass_utils import run_bass_kernel_spmd

    nc = _get_program()
    return run_bass_kernel_spmd(
        nc, in_maps, core_ids=list(range(N_CORES)), **kwargs
    )


def kernel(x: np.ndarray, weights: np.ndarray, bias: np.ndarray) -> np.ndarray:
    x = np.asarray(x, dtype=np.float32)
    weights = np.asarray(weights, dtype=np.float32)
    bias = np.asarray(bias, dtype=np.float32)
    in_maps = _shard_inputs(x, weights, bias)
    res = run_sharded(in_maps)
    full = np.empty((B, IO), dtype=np.float32)
    for c in range(N_CORES):
        sl = slice(c * N_SHARD, (c + 1) * N_SHARD)
        full[:, sl] = np.asarray(res.results[c]["out"]).astype(np.float32).T
    return full.reshape(B, IN_SIZE, OUT_SIZE)
